# revision 1
# baseline (speedup 1.0000x reference)
"""BlockCrossAttention TRN2 Bass kernel — 8-core SPMD, no collectives.

Sharding: core c => batch b = c//4, block-quarter q = c%4.
Each core: pools its 2048 decoder tokens into 128 blocks, computes full
K/V for its batch (all 4 kv heads), runs attention for all 16 q-heads
over its 128 blocks, output-projects, and writes block-level output rows
[128, 1024].  Host broadcasts block rows back to token level (pure
replication) and concatenates.

Numerics: projections in bf16 (K/V/Q) and f32r (out-proj); softmax in
f32 (exp on ACT), attention weights bf16.  Mask folded into V and the
denominator (masked enc tokens contribute exactly 0, matching the
reference's exp(-1e9) == 0).
"""
import os
import sys

sys.path.insert(0, "/opt/trn_rl_repo")

DEBUG_PHASE = int(os.environ.get("KDBG_PHASE", "9"))

import numpy as np

import concourse.bass as bass
import concourse.tile as tile
from concourse import bacc, mybir
from concourse.bass import ts
from concourse.bass_utils import run_bass_kernel_spmd
from concourse.masks import make_identity

F32 = mybir.dt.float32
F32R = mybir.dt.float32r
BF16 = mybir.dt.bfloat16
I32 = mybir.dt.int32

# problem constants (hardcoded per contract)
B, LDEC, LENC, D = 2, 8192, 4096, 1024
BLOCK, H, KV, DH = 16, 16, 4, 64
NB = LDEC // BLOCK            # 512 blocks per batch
NCORES = 8
TOK = LDEC // 4               # 2048 decoder tokens per core
NBQ = NB // 4                 # 128 blocks per core
# pooled is a SUM over 16 tokens (not mean); fold /16 into the exp scale
SCALE = float(1.0 / (np.sqrt(np.float32(DH)).astype(np.float32) * BLOCK))

_CACHE = {}


def _build():
    nc = bacc.Bacc("TRN2", target_bir_lowering=False, debug=False,
                   num_devices=NCORES)
    hs = nc.dram_tensor("hs", [TOK, D], F32, kind="ExternalInput").ap()
    encT = nc.dram_tensor("encT", [D, LENC], F32, kind="ExternalInput").ap()
    maskpm = nc.dram_tensor("maskpm", [128, 32], I32, kind="ExternalInput").ap()
    wq = nc.dram_tensor("wq", [D, H * DH], F32, kind="ExternalInput").ap()
    wk = nc.dram_tensor("wk", [D, KV * DH], F32, kind="ExternalInput").ap()
    wv = nc.dram_tensor("wv", [D, KV * DH], F32, kind="ExternalInput").ap()
    wo = nc.dram_tensor("wo", [H * DH, D], F32, kind="ExternalInput").ap()
    outb = nc.dram_tensor("outb", [NBQ, D], F32, kind="ExternalOutput").ap()

    with tile.TileContext(nc) as tc:
        _body(nc, tc, hs, encT, maskpm, wq, wk, wv, wo, outb)
    nc.compile()
    return nc


def _body(nc, tc, hs, encT, maskpm, wq, wk, wv, wo, outb):
    EC = 32          # enc chunks of 128 tokens
    KD = 8           # 128-wide chunks of D

    from contextlib import ExitStack
    with ExitStack() as ctx:
        pool = lambda name, bufs, **kw: ctx.enter_context(
            tc.tile_pool(name=name, bufs=bufs, **kw))
        # ---- long-lived pools ----
        constp = pool("const", 1)
        ktp = pool("kt", 8)
        v5p = pool("v5", EC)
        qpp = pool("qp", 1)
        otp = pool("ot", 1)
        wbf = pool("wbf", KD)
        small = pool("small", 4)

        # ---- constants ----
        ident = constp.tile([128, 128], F32)
        make_identity(nc, ident[:])
        maskstage = small.tile([128, 32], I32)
        nc.sync.dma_start(maskstage[:], maskpm[:])
        maskf = constp.tile([128, 32], F32)
        nc.vector.tensor_copy(maskf[:], maskstage[:])
        maskbf = constp.tile([128, 32], BF16)
        nc.vector.tensor_copy(maskbf[:], maskstage[:])

        # ---- pooling: pooled[p, d] = sum_j hs[16p + j, d] ----
        pooled = constp.tile([128, D], F32)
        hsr = hs.rearrange("(p j) d -> p j d", j=BLOCK)
        with tc.tile_pool(name="jbig", bufs=2) as jbig, \
             tc.tile_pool(name="padd", bufs=1) as padd:
            j0 = jbig.tile([128, 8 * D], F32, tag="jb")
            nc.sync.dma_start(j0[:].rearrange("p (j d) -> p j d", d=D),
                              hsr[:, 0:8, :])
            j1 = jbig.tile([128, 8 * D], F32, tag="jb")
            nc.sync.dma_start(j1[:].rearrange("p (j d) -> p j d", d=D),
                              hsr[:, 8:16, :])
            s1 = padd.tile([128, 8 * D], F32, tag="s1")
            nc.vector.tensor_add(s1[:], j0[:], j1[:])
            s2 = padd.tile([128, 4 * D], F32, tag="s2")
            nc.vector.tensor_add(s2[:], s1[:, 0:4 * D], s1[:, 4 * D:8 * D])
            s3 = padd.tile([128, 2 * D], F32, tag="s3")
            nc.vector.tensor_add(s3[:], s2[:, 0:2 * D], s2[:, 2 * D:4 * D])
            nc.vector.tensor_add(pooled[:], s3[:, 0:D], s3[:, D:2 * D])

        # ---- weights: load + cast ----
        wq_bf, wk_bf, wv_bf = [], [], []
        with tc.tile_pool(name="wstage", bufs=2) as wstage:
            for k in range(KD):
                st = wstage.tile([128, H * DH], F32, tag="wqs")
                nc.sync.dma_start(st[:], wq[ts(k, 128), :])
                bq = wbf.tile([128, H * DH], BF16, tag="wqb")
                nc.gpsimd.tensor_copy(bq[:], st[:])
                wq_bf.append(bq)
            for k in range(KD):
                st = wstage.tile([128, KV * DH], F32, tag="wks")
                nc.sync.dma_start(st[:], wk[ts(k, 128), :])
                bk = wbf.tile([128, KV * DH], BF16, tag="wkb")
                nc.gpsimd.tensor_copy(bk[:], st[:])
                wk_bf.append(bk)
            for k in range(KD):
                st = wstage.tile([128, KV * DH], F32, tag="wvs")
                nc.sync.dma_start(st[:], wv[ts(k, 128), :])
                bv = wbf.tile([128, KV * DH], BF16, tag="wvb")
                nc.gpsimd.tensor_copy(bv[:], st[:])
                wv_bf.append(bv)

        # ---- transpose pooled + Q^T projection -> packed tiles ----
        qpack = [[qpp.tile([128, NBQ], BF16, tag=f"qp{mm}{j}", name=f"qp{mm}{j}")
                  for j in range(4)] for mm in range(2)]
        with tc.tile_pool(name="tpt", bufs=KD) as tptp, \
             tc.tile_pool(name="pt", bufs=2, space="PSUM") as ppt, \
             tc.tile_pool(name="pq", bufs=2, space="PSUM") as ppq:
            tpT = []
            for k in range(KD):
                ps = ppt.tile([128, 128], F32)
                nc.tensor.transpose(ps[:], pooled[:, ts(k, 128)], ident[:])
                tb = tptp.tile([128, 128], BF16, tag="tpT")
                nc.vector.tensor_copy(tb[:], ps[:])
                tpT.append(tb)
            for m in range(8):
                ps = ppq.tile([128, NBQ], F32)
                for k in range(KD):
                    nc.tensor.matmul(ps[:], wq_bf[k][:, ts(m, 128)], tpT[k][:],
                                     start=(k == 0), stop=(k == KD - 1))
                for half in range(2):
                    h = 2 * m + half
                    mm, j, dsthalf = h // 8, h % 4, (h % 8) // 4
                    nc.vector.tensor_copy(
                        qpack[mm][j][ts(dsthalf, 64), :], ps[ts(half, 64), :])

        # ---- enc^T load + cast, then K^T and V (enc scoped) ----
        KT = [[None] * 8 for _ in range(2)]
        V5 = []
        with tc.tile_pool(name="estage", bufs=2) as estage, \
             tc.tile_pool(name="encbf", bufs=KD) as encbf, \
             tc.tile_pool(name="pk", bufs=2, space="PSUM") as ppk, \
             tc.tile_pool(name="pv", bufs=2, space="PSUM") as ppv:
            enc_bf = []
            for k in range(KD):
                st = estage.tile([128, LENC], F32)
                nc.sync.dma_start(st[:], encT[ts(k, 128), :])
                eb = encbf.tile([128, LENC], BF16, tag="encbf")
                nc.gpsimd.tensor_copy(eb[:], st[:])
                enc_bf.append(eb)
            for mk in range(2):
                for ce in range(8):
                    ps = ppk.tile([128, 512], F32)
                    for k in range(KD):
                        nc.tensor.matmul(ps[:], wk_bf[k][:, ts(mk, 128)],
                                         enc_bf[k][:, ts(ce, 512)],
                                         start=(k == 0), stop=(k == KD - 1))
                    tb = ktp.tile([128, 512], BF16, tag=f"kt{mk}",
                                  name=f"kt{mk}_{ce}")
                    nc.vector.tensor_copy(tb[:], ps[:])
                    KT[mk][ce] = tb
            # V (natural, masked): V5[c] [128, 4*65]
            for c in range(EC):
                ps = ppv.tile([128, KV * DH], F32)
                for k in range(KD):
                    nc.tensor.matmul(ps[:], enc_bf[k][:, ts(c, 128)],
                                     wv_bf[k][:],
                                     start=(k == 0), stop=(k == KD - 1))
                t5 = v5p.tile([128, KV * (DH + 1)], BF16, tag="v5",
                              name=f"v5_{c}")
                t5r = t5[:].rearrange("p (g x) -> p g x", x=DH + 1)
                psr = ps[:].rearrange("p (g x) -> p g x", x=DH)
                nc.vector.tensor_scalar_mul(t5r[:, :, 0:DH], psr,
                                            maskf[:, c:c + 1])
                nc.vector.tensor_copy(
                    t5r[:, :, DH:DH + 1],
                    maskbf[:, c:c + 1].broadcast_to((128, KV, 1)))
                V5.append(t5)

        # ---- attention: head pairs (8mm+j, 8mm+4+j), kv pair (2mm, 2mm+1) ----
        OT = [otp.tile([128, NBQ], BF16, tag=f"ot{t}", name=f"ot{t}")
              for t in range(8)]
        wo_sb = []
        with tc.tile_pool(name="wop", bufs=KD) as wop, \
             tc.tile_pool(name="attn", bufs=4) as attnp, \
             tc.tile_pool(name="psc", bufs=2, space="PSUM") as ppsc, \
             tc.tile_pool(name="pav", bufs=1, space="PSUM") as ppav:
            for t in range(8):
                st = wop.tile([128, D], F32, tag="wos", name=f"wos{t}")
                nc.sync.dma_start(st[:], wo[ts(t, 128), :])
                bb = wop.tile([128, D], BF16, tag="wo", name=f"wo{t}")
                nc.gpsimd.tensor_copy(bb[:], st[:])
                wo_sb.append(bb)
            if DEBUG_PHASE < 2:
                for t in range(8):
                    nc.vector.memset(OT[t][:], 0.001)
            for mm in range(2 if DEBUG_PHASE >= 2 else 0):
                for j in range(4):
                    hA, hB = 8 * mm + j, 8 * mm + 4 + j
                    gA, gB = 2 * mm, 2 * mm + 1
                    avA = ppav.tile([DH + 1, NBQ], F32, tag="avA")
                    avB = ppav.tile([DH + 1, NBQ], F32, tag="avB")
                    for c0 in range(8):
                        scA = ppsc.tile([128, 512], F32, tag="scA")
                        scB = ppsc.tile([128, 512], F32, tag="scB")
                        for i in range(4):
                            c = 4 * c0 + i
                            lhs = KT[mm][c // 4][:, ts(c % 4, 128)]
                            nc.tensor.matmul(
                                scA[:, ts(i, 128)], lhs[0:64, :],
                                qpack[mm][j][0:64, :],
                                start=True, stop=True)
                            nc.tensor.matmul(
                                scB[:, ts(i, 128)], lhs[64:128, :],
                                qpack[mm][j][64:128, :],
                                start=True, stop=True)
                        eA = attnp.tile([128, 512], BF16, tag="eA")
                        eB = attnp.tile([128, 512], BF16, tag="eB")
                        nc.scalar.activation(eA[:], scA[:],
                                             mybir.ActivationFunctionType.Exp,
                                             bias=0.0, scale=SCALE)
                        nc.scalar.activation(eB[:], scB[:],
                                             mybir.ActivationFunctionType.Exp,
                                             bias=0.0, scale=SCALE)
                        for i in range(4 if DEBUG_PHASE >= 3 else 0):
                            c = 4 * c0 + i
                            nc.tensor.matmul(
                                avA[:], V5[c][:, ts(gA, DH + 1)],
                                eA[:, ts(i, 128)],
                                start=(c == 0), stop=(c == EC - 1))
                            nc.tensor.matmul(
                                avB[:], V5[c][:, ts(gB, DH + 1)],
                                eB[:, ts(i, 128)],
                                start=(c == 0), stop=(c == EC - 1))
                        if DEBUG_PHASE == 2 and c0 == 7:
                            nc.tensor.matmul(avA[:], V5[0][:, ts(gA, DH + 1)],
                                             eA[:, ts(0, 128)],
                                             start=True, stop=True)
                            nc.tensor.matmul(avB[:], V5[0][:, ts(gB, DH + 1)],
                                             eB[:, ts(0, 128)],
                                             start=True, stop=True)
                    for h, av in ((hA, avA), (hB, avB)):
                        rec = small.tile([1, NBQ], F32, tag="rec")
                        nc.vector.reciprocal(rec[:], av[DH:DH + 1, :])
                        recb = small.tile([DH, NBQ], F32, tag="recb")
                        nc.gpsimd.partition_broadcast(recb[:], rec[:])
                        dst = OT[h // 2][ts(h % 2, 64), :]
                        nc.vector.tensor_mul(dst, av[0:DH, :], recb[:])

            # ---- out projection ----
            with tc.tile_pool(name="outsb", bufs=1) as outsbp, \
                 tc.tile_pool(name="po", bufs=2, space="PSUM") as ppo:
                osb = outsbp.tile([128, D], F32)
                for n in range(2):
                    ps = ppo.tile([128, 512], F32)
                    for t in range(8):
                        nc.tensor.matmul(ps[:], OT[t][:],
                                         wo_sb[t][:, ts(n, 512)],
                                         start=(t == 0), stop=(t == 7))
                    nc.vector.tensor_copy(osb[:, ts(n, 512)], ps[:])
                nc.sync.dma_start(outb[:], osb[:])


def kernel(hidden_states, encoder_hidden_states, attention_mask, Wq, Wk, Wv, Wo):
    if "nc" not in _CACHE:
        _CACHE["nc"] = _build()
    nc = _CACHE["nc"]

    hidden_states = np.ascontiguousarray(hidden_states, dtype=np.float32)
    enc = np.ascontiguousarray(encoder_hidden_states, dtype=np.float32)
    mask = np.asarray(attention_mask, dtype=np.int32)
    in_maps = []
    for c in range(NCORES):
        b, q = c // 4, c % 4
        in_maps.append({
            "hs": np.ascontiguousarray(hidden_states[b, q * TOK:(q + 1) * TOK]),
            "encT": np.ascontiguousarray(enc[b].T),
            "maskpm": np.ascontiguousarray(mask[b].reshape(32, 128).T),
            "wq": np.ascontiguousarray(Wq, dtype=np.float32),
            "wk": np.ascontiguousarray(Wk, dtype=np.float32),
            "wv": np.ascontiguousarray(Wv, dtype=np.float32),
            "wo": np.ascontiguousarray(Wo, dtype=np.float32),
        })
    res = run_bass_kernel_spmd(nc, in_maps, list(range(NCORES)),
                               **_CACHE.get("run_kwargs", {}))
    _CACHE["last_result"] = res
    blocks = np.empty((B, NB, D), dtype=np.float32)
    for c in range(NCORES):
        b, q = c // 4, c % 4
        blocks[b, q * NBQ:(q + 1) * NBQ] = res.results[c]["outb"]
    out = np.repeat(blocks, BLOCK, axis=1)
    return out



# revision 12
# speedup vs baseline: 2.6425x; 2.6425x over previous
"""BlockCrossAttention TRN2 Bass kernel — 8-core SPMD with KV AllGather.

Sharding: core c => batch b = c//4, quarter r = c%4.
Host side: compacts encoder tokens by the attention mask (~2056 of 4096
kept; capacity CAP=2304), transposes hs/enc slices, and pre-casts all
matmul operands to bf16.

Per core:
  - pool_avg its 2048 decoder tokens -> pooledT [D, 128 blocks] (bf16)
  - project K^T,V for its OWN quarter of compacted enc tokens (576)
  - 2x AllGather (bf16, ~0.3 MiB in) across the 4 cores of its batch
    -> full K^T [256hd, 2304], V [2304, 256hd]
  - attention for all 16 heads over its own 128 blocks (softmax via a
    mask column appended to V: denominator excludes pad tokens)
  - local out-projection with full Wo -> [128 blocks, 1024]
Host broadcasts block rows back to token level and concatenates.

Numerics: all matmuls bf16 (inputs host-cast), accumulation f32 in
PSUM, exp on ACT in f32->bf16.  Compaction is exact: reference's
masked scores give exp(-1e9)==0 contributions.
"""
import sys

sys.path.insert(0, "/opt/trn_rl_repo")

import numpy as np
import ml_dtypes

import concourse.bass as bass
import concourse.tile as tile
from concourse import bacc, mybir
from concourse.bass import ts
from concourse.bass_utils import run_bass_kernel_spmd

F32 = mybir.dt.float32
BF16 = mybir.dt.bfloat16

# problem constants (hardcoded per contract)
B, LDEC, LENC, D = 2, 8192, 4096, 1024
BLOCK, H, KV, DH = 16, 16, 4, 64
NB = LDEC // BLOCK            # 512 blocks per batch
NCORES = 8
TOK = LDEC // 4               # 2048 decoder tokens per core
NBQ = NB // 4                 # 128 blocks per core
KD = D // 128                 # 8 chunks of D
CAP = 2304                    # compacted-enc capacity (18 * 128)
CAPQ = CAP // 4               # 576 enc tokens projected per core
EC = CAP // 128               # 18 chunks of 128 enc tokens
HALF = EC // 2                # 9 chunks per exp batch
# pooled is a SUM over 16 tokens (add tree); fold the /16 into the exp scale
SCALE = float(1.0 / (np.sqrt(np.float32(DH)) * BLOCK))

_CACHE = {}
BF = ml_dtypes.bfloat16


def _build():
    nc = bacc.Bacc("TRN2", target_bir_lowering=False, debug=False,
                   num_devices=NCORES)
    hsT = nc.dram_tensor("hsT", [D, TOK], BF16, kind="ExternalInput").ap()
    encTq = nc.dram_tensor("encTq", [D, CAPQ], BF16, kind="ExternalInput").ap()
    maskpm = nc.dram_tensor("maskpm", [128, EC], BF16, kind="ExternalInput").ap()
    wq = nc.dram_tensor("wq", [D, H * DH], BF16, kind="ExternalInput").ap()
    wk = nc.dram_tensor("wk", [D, KV * DH], BF16, kind="ExternalInput").ap()
    wv = nc.dram_tensor("wv", [D, KV * DH], BF16, kind="ExternalInput").ap()
    wo = nc.dram_tensor("wo", [H * DH, D], BF16, kind="ExternalInput").ap()
    outb = nc.dram_tensor("outb", [NBQ, D], F32, kind="ExternalOutput").ap()

    with tile.TileContext(nc) as tc:
        _body(nc, tc, hsT, encTq, maskpm, wq, wk, wv, wo, outb)
    nc.compile()
    return nc


def _body(nc, tc, hsT, encTq, maskpm, wq, wk, wv, wo, outb):
    from contextlib import ExitStack
    groups = [[0, 1, 2, 3], [4, 5, 6, 7]]
    with ExitStack() as ctx:
        pool = lambda name, bufs, **kw: ctx.enter_context(
            tc.tile_pool(name=name, bufs=bufs, **kw))
        constp = pool("const", 1)
        wkvp = pool("wkv", KD)
        encp = pool("enc", KD)
        wqp = pool("wq", KD)
        wop = pool("wo", KD)
        pooledp = pool("pooled", KD)
        qtp = pool("qt", KD)
        ktp = pool("ktsb", 1)
        v5p = pool("v5", EC)
        otp = pool("ot", 1)
        smallp = pool("small", 8)
        dramp = pool("dram", 1, space="DRAM")

        # ---- small consts / kv weights / own-quarter enc ----
        maskbf = constp.tile([128, EC], BF16)
        nc.sync.dma_start(maskbf[:], maskpm[:])
        wk_sb, wv_sb, enc_sb = [], [], []
        for k in range(KD):
            t = wkvp.tile([128, KV * DH], BF16, tag="wk", name=f"wk{k}")
            nc.sync.dma_start(t[:], wk[ts(k, 128), :])
            wk_sb.append(t)
        for k in range(KD):
            t = wkvp.tile([128, KV * DH], BF16, tag="wv", name=f"wv{k}")
            nc.sync.dma_start(t[:], wv[ts(k, 128), :])
            wv_sb.append(t)
        for k in range(KD):
            t = encp.tile([128, CAPQ], BF16, tag="enc", name=f"enc{k}")
            nc.sync.dma_start(t[:], encTq[ts(k, 128), :])
            enc_sb.append(t)

        # ---- K^T/V projection on own quarter, staged to DRAM ----
        kt_in = dramp.tile([2 * 128, CAPQ], BF16, name="kt_in")
        v_in = dramp.tile([CAPQ, KV * DH], BF16, name="v_in")
        kt_out = nc.dram_tensor("kt_out", [8 * 128, CAPQ], BF16).ap()
        v_out = nc.dram_tensor("v_out", [CAP, KV * DH], BF16).ap()

        with tc.tile_pool(name="pk", bufs=2, space="PSUM") as ppk, \
             tc.tile_pool(name="pv", bufs=2, space="PSUM") as ppv, \
             tc.tile_pool(name="kvst", bufs=4) as kvst:
            for mk in range(2):
                ps = ppk.tile([128, CAPQ], F32, tag="pk")
                for k in range(KD):
                    for n0, nw in ((0, 512), (512, CAPQ - 512)):
                        nc.tensor.matmul(ps[:, n0:n0 + nw],
                                         wk_sb[k][:, ts(mk, 128)],
                                         enc_sb[k][:, n0:n0 + nw],
                                         start=(k == 0), stop=(k == KD - 1))
                kst = kvst.tile([128, CAPQ], BF16, tag="kst")
                nc.vector.tensor_copy(kst[:], ps[:])
                nc.sync.dma_start(kt_in[ts(mk, 128), :], kst[:])
            for c5 in range(5):
                p0 = 128 * c5
                pw = min(128, CAPQ - p0)
                ps = ppv.tile([128, KV * DH], F32, tag="pv")
                for k in range(KD):
                    nc.tensor.matmul(ps[0:pw, :], enc_sb[k][:, p0:p0 + pw],
                                     wv_sb[k][:],
                                     start=(k == 0), stop=(k == KD - 1))
                vst = kvst.tile([128, KV * DH], BF16, tag="vst")
                nc.vector.tensor_copy(vst[0:pw, :], ps[0:pw, :])
                nc.sync.dma_start(v_in[p0:p0 + pw, :], vst[0:pw, :])

        # ---- AllGather K^T and V across the 4 cores of this batch ----
        nc.gpsimd.collective_compute(
            "AllGather", mybir.AluOpType.bypass, replica_groups=groups,
            ins=[kt_in[:].opt()], outs=[kt_out.opt()])
        nc.gpsimd.collective_compute(
            "AllGather", mybir.AluOpType.bypass, replica_groups=groups,
            ins=[v_in[:].opt()], outs=[v_out.opt()])

        # ---- pooling + Q^T projection (independent chain) ----
        with tc.tile_pool(name="hst", bufs=3) as hstp, \
             tc.tile_pool(name="padd", bufs=2) as paddp, \
             tc.tile_pool(name="pq", bufs=2, space="PSUM") as ppq:
            pooledT = []
            for k in range(KD):
                ht = hstp.tile([128, TOK], BF16, tag="hst")
                nc.sync.dma_start(ht[:], hsT[ts(k, 128), :])
                htr = ht[:].rearrange("p (b j) -> p b j", j=BLOCK)
                s1 = paddp.tile([128, NBQ * 8], F32, tag="s1")
                s1r = s1[:].rearrange("p (b j) -> p b j", j=8)
                nc.vector.tensor_add(s1r, htr[:, :, 0:8], htr[:, :, 8:16])
                s2 = paddp.tile([128, NBQ * 4], F32, tag="s2")
                s2r = s2[:].rearrange("p (b j) -> p b j", j=4)
                nc.vector.tensor_add(s2r, s1r[:, :, 0:4], s1r[:, :, 4:8])
                s3 = paddp.tile([128, NBQ * 2], F32, tag="s3")
                s3r = s3[:].rearrange("p (b j) -> p b j", j=2)
                nc.vector.tensor_add(s3r, s2r[:, :, 0:2], s2r[:, :, 2:4])
                pt = pooledp.tile([128, NBQ], BF16, tag="pt", name=f"pt{k}")
                ptr = pt[:].rearrange("p (b j) -> p b j", j=1)
                nc.vector.tensor_add(ptr, s3r[:, :, 0:1], s3r[:, :, 1:2])
                pooledT.append(pt)
            wq_sb = []
            for k in range(KD):
                t = wqp.tile([128, H * DH], BF16, tag="wq", name=f"wq{k}")
                nc.sync.dma_start(t[:], wq[ts(k, 128), :])
                wq_sb.append(t)
            # qpack[mm][j]: rows 0:64 = q^T of head 8mm+j, rows 64:128 = head
            # 8mm+4+j (pairs heads whose kv groups are 2mm / 2mm+1 so the
            # base partitions line up with the K^T pair tiles).
            qpack = [[qtp.tile([128, NBQ], BF16, tag=f"qp{mm}{j}",
                               name=f"qp{mm}{j}") for j in range(4)]
                     for mm in range(2)]
            for m in range(KD):
                ps = ppq.tile([128, NBQ], F32, tag="pq")
                for k in range(KD):
                    nc.tensor.matmul(ps[:], wq_sb[k][:, ts(m, 128)],
                                     pooledT[k][:],
                                     start=(k == 0), stop=(k == KD - 1))
                for half in range(2):
                    h = 2 * m + half
                    mm, j, dsthalf = h // 8, h % 4, (h % 8) // 4
                    nc.vector.tensor_copy(
                        qpack[mm][j][ts(dsthalf, 64), :], ps[ts(half, 64), :])

        # wo loads (needed only at the end)
        wo_sb = []
        for t in range(KD):
            tl = wop.tile([128, D], BF16, tag="wo", name=f"wo{t}")
            nc.sync.dma_start(tl[:], wo[ts(t, 128), :])
            wo_sb.append(tl)

        # ---- assemble K^T pair tiles and V5 (V + mask column) ----
        # KT_sb[mk]: rows 0:64 = group 2mk, rows 64:128 = group 2mk+1
        # (hd = g*64+dh = mk*128 + row, so this is a contiguous row slice
        # of each rank's chunk in the AG output).
        KT_sb = []
        for mk in range(2):
            kt = ktp.tile([128, CAP], BF16, tag=f"ktg{mk}", name=f"ktg{mk}")
            nc.sync.dma_start(
                kt[:].rearrange("p (r t) -> p r t", r=4),
                kt_out.rearrange("(r h) t -> h r t", r=4)[ts(mk, 128), :, :])
            KT_sb.append(kt)
        V5 = []
        v_outr = v_out.rearrange("(c p) (g d) -> c p g d", p=128, d=DH)
        for c in range(EC):
            t5 = v5p.tile([128, KV * (DH + 1)], BF16, tag="v5", name=f"v5_{c}")
            t5r = t5[:].rearrange("p (g x) -> p g x", x=DH + 1)
            nc.sync.dma_start(t5r[:, :, 0:DH], v_outr[c])
            nc.vector.tensor_copy(
                t5r[:, :, DH:DH + 1],
                maskbf[:, c:c + 1].broadcast_to((128, KV, 1)))
            V5.append(t5)

        # ---- attention: per head, exp in 2 batches of 9 chunks ----
        OT = [otp.tile([128, NBQ], BF16, tag=f"ot{t}", name=f"ot{t}")
              for t in range(KD)]
        with tc.tile_pool(name="attn", bufs=4) as attnp, \
             tc.tile_pool(name="psc", bufs=1, space="PSUM") as ppsc, \
             tc.tile_pool(name="pav", bufs=1, space="PSUM") as ppav:
            for mm in range(2):
                for j in range(4):
                    hA, hB = 8 * mm + j, 8 * mm + 4 + j
                    gA, gB = 2 * mm, 2 * mm + 1
                    # avA/avB must live in SEPARATE psum banks: a start=True
                    # matmul clears the whole bank's accumulation state, so
                    # interleaved open accumulation groups cannot share one.
                    avAt = ppav.tile([DH + 1, NBQ], F32, tag="avA")
                    avBt = ppav.tile([DH + 1, NBQ], F32, tag="avB")
                    avA, avB = avAt[:], avBt[:]
                    for half in range(2):
                        scA = ppsc.tile([128, HALF * 128], F32, tag="scA")
                        scB = ppsc.tile([128, HALF * 128], F32, tag="scB")
                        for i in range(HALF):
                            c = half * HALF + i
                            lhs = KT_sb[mm][:, ts(c, 128)]
                            nc.tensor.matmul(scA[:, ts(i, 128)], lhs[0:64, :],
                                             qpack[mm][j][0:64, :],
                                             start=True, stop=True)
                            nc.tensor.matmul(scB[:, ts(i, 128)], lhs[64:128, :],
                                             qpack[mm][j][64:128, :],
                                             start=True, stop=True)
                        eA = attnp.tile([128, HALF * 128], BF16, tag="eA")
                        eB = attnp.tile([128, HALF * 128], BF16, tag="eB")
                        nc.scalar.activation(eA[:], scA[:],
                                             mybir.ActivationFunctionType.Exp,
                                             bias=0.0, scale=SCALE)
                        nc.scalar.activation(eB[:], scB[:],
                                             mybir.ActivationFunctionType.Exp,
                                             bias=0.0, scale=SCALE)
                        for i in range(HALF):
                            c = half * HALF + i
                            nc.tensor.matmul(avA, V5[c][:, ts(gA, DH + 1)],
                                             eA[:, ts(i, 128)],
                                             start=(c == 0), stop=(c == EC - 1))
                            nc.tensor.matmul(avB, V5[c][:, ts(gB, DH + 1)],
                                             eB[:, ts(i, 128)],
                                             start=(c == 0), stop=(c == EC - 1))
                    for h, av in ((hA, avA), (hB, avB)):
                        rec = smallp.tile([1, NBQ], F32, tag="rec")
                        nc.vector.reciprocal(rec[:], av[DH:DH + 1, :])
                        recb = smallp.tile([DH, NBQ], F32, tag="recb")
                        nc.gpsimd.partition_broadcast(recb[:], rec[:])
                        nc.vector.tensor_mul(OT[h // 2][ts(h % 2, 64), :],
                                             av[0:DH, :], recb[:])

        # ---- out projection (local, full Wo) ----
        with tc.tile_pool(name="outsb", bufs=1) as outsbp, \
             tc.tile_pool(name="po", bufs=2, space="PSUM") as ppo:
            osb = outsbp.tile([128, D], F32)
            for n in range(2):
                ps = ppo.tile([128, 512], F32)
                for t in range(KD):
                    nc.tensor.matmul(ps[:], OT[t][:], wo_sb[t][:, ts(n, 512)],
                                     start=(t == 0), stop=(t == KD - 1))
                nc.vector.tensor_copy(osb[:, ts(n, 512)], ps[:])
            nc.sync.dma_start(outb[:], osb[:])


def kernel(hidden_states, encoder_hidden_states, attention_mask, Wq, Wk, Wv, Wo):
    if "nc" not in _CACHE:
        _CACHE["nc"] = _build()
    nc = _CACHE["nc"]

    hs = np.asarray(hidden_states, dtype=np.float32)
    enc = np.asarray(encoder_hidden_states, dtype=np.float32)
    mask = np.asarray(attention_mask)

    # host-side compaction (exact: masked tokens contribute exp(-1e9)==0)
    encT_c, maskpm_b = [], []
    for b in range(B):
        idx = np.nonzero(mask[b])[0]
        assert len(idx) <= CAP, f"mask keeps {len(idx)} > CAP={CAP} tokens"
        ec = np.zeros((CAP, D), dtype=np.float32)
        ec[:len(idx)] = enc[b][idx]
        encT_c.append(np.ascontiguousarray(ec.T.astype(BF)))
        mc = np.zeros((CAP,), dtype=np.float32)
        mc[:len(idx)] = 1.0
        maskpm_b.append(np.ascontiguousarray(mc.reshape(EC, 128).T.astype(BF)))

    wq_b = np.ascontiguousarray(np.asarray(Wq, np.float32).astype(BF))
    wk_b = np.ascontiguousarray(np.asarray(Wk, np.float32).astype(BF))
    wv_b = np.ascontiguousarray(np.asarray(Wv, np.float32).astype(BF))
    wo_b = np.ascontiguousarray(np.asarray(Wo, np.float32).astype(BF))

    in_maps = []
    for c in range(NCORES):
        b, r = c // 4, c % 4
        in_maps.append({
            "hsT": np.ascontiguousarray(
                hs[b, r * TOK:(r + 1) * TOK].T.astype(BF)),
            "encTq": np.ascontiguousarray(
                encT_c[b][:, r * CAPQ:(r + 1) * CAPQ]),
            "maskpm": maskpm_b[b],
            "wq": wq_b,
            "wk": wk_b,
            "wv": wv_b,
            "wo": wo_b,
        })
    res = run_bass_kernel_spmd(nc, in_maps, list(range(NCORES)),
                               **_CACHE.get("run_kwargs", {}))
    _CACHE["last_result"] = res
    blocks = np.empty((B, NB, D), dtype=np.float32)
    for c in range(NCORES):
        b, r = c // 4, c % 4
        blocks[b, r * NBQ:(r + 1) * NBQ] = res.results[c]["outb"]
    out = np.repeat(blocks, BLOCK, axis=1)
    return out
